# revision 29
# baseline (speedup 1.0000x reference)
"""Trainium2 Bass kernel for nn_CapacitanceMatrix.

C[b, i, j] = sigmoid(x[b]·Wd[i] + bd[i])        if i == j
           = -softplus(x[b]·Wo[m] + bo[m])      if i != j  (m = row-major off-diag idx)

Strategy: fold the scatter into the weight layout. Build W_full (256, D) whose
row p = i*16+j is Wd[i] (diag) or Wo[m] (off-diag), so the matmul output row is
already the flattened (16, 16) matrix. Pure data parallel over 8 cores: each
core gets 8192 rows of x, pre-transposed on host to xT (D, 8192) so the
contraction dim sits on SBUF partitions with contiguous DMA.

Per core: for each 128-row batch tile, accumulate 8 matmuls over D-chunks
(lhsT = xT chunk stationary, rhs = W_full^T (128, 256) moving) into PSUM after
seeding it with the bias via a K=1 ones x bias matmul. Epilogue: softplus
(ScalarE LUT) -> negate (VectorE) -> sigmoid overwrite of the 16 diagonal
columns (stride-17 AP). Output rows DMA out contiguously.
"""

import os
import sys

sys.path.insert(0, "/opt/trn_rl_repo")

from contextlib import ExitStack

import numpy as np

import concourse.bass as bass
import concourse.tile as tile
from concourse import bacc, mybir
from concourse.bass_utils import run_bass_kernel_spmd

B = 65536
D = 1024
K = 16
NOUT = K * K  # 256
NCORES = 8
BC = B // NCORES  # 8192 rows per core
KD = D // 128  # 8 contraction chunks
BLK = 1024  # batch columns loaded per block
OJ = 4  # j-subtiles batched per output DMA
CONST_F = KD * NOUT + 2 * NOUT + 128  # wt chunks + doubled bias + ones

# matmul dtype: "float32r" (fp32 storage, full-rate PE at N>=256) or "bfloat16"
MM_DT_NAME = os.environ.get("CAP_MM_DT", "float32r")

_CACHE = {}

_ACT_TABLES_PATCHED = False


def _pin_act_table_set():
    """Force Exp and Ln to resolve to the single LUT set that holds both
    (`natural_log_exp_and_others`), so the epilogue's exp->ln alternation
    doesn't thrash ACT_TABLE_LOADs (measured: 54 loads, 69us of ScalarE)."""
    global _ACT_TABLES_PATCHED
    if _ACT_TABLES_PATCHED:
        return
    import concourse.hw_specs as hw_specs

    orig = hw_specs.get_activation_tables

    def patched(arch):
        tables = {k: set(v) for k, v in orig(arch).items()}
        keep = "natural_log_exp_and_others"
        if keep in tables:
            for k, v in tables.items():
                if k != keep:
                    v.discard(mybir.ActivationFunctionType.Exp)
                    v.discard(mybir.ActivationFunctionType.Ln)
        return tables

    bacc.get_activation_tables = patched
    _ACT_TABLES_PATCHED = True


def _mm_dt():
    return getattr(mybir.dt, MM_DT_NAME)


def _np_dt():
    return mybir.dt.np(_mm_dt())


def _build_bass():
    _pin_act_table_set()
    mm_dt = _mm_dt()
    f32 = mybir.dt.float32
    nc = bacc.Bacc("TRN2", target_bir_lowering=False, debug=False)
    # x pre-tiled on host: [block, chunk, partition, col] so every chunk-block
    # DMA is one fully contiguous DRAM read
    xT = nc.dram_tensor(
        "xT", [BC // BLK, KD, 128, BLK], mm_dt, kind="ExternalInput"
    ).ap()
    # single const blob (one DMA -> one semaphore: the LDWEIGHTS half of a
    # self-loading matmul only has ONE sync-wait slot, so all constants must
    # arrive behind a single sem): cols 0:2048 = wt chunks, row 0 extras:
    # 2048:2304 = bias, 2304:2432 = ones
    consts = nc.dram_tensor("consts", [128, CONST_F], mm_dt, kind="ExternalInput").ap()
    out = nc.dram_tensor("out", [BC, NOUT], f32, kind="ExternalOutput").ap()

    with tile.TileContext(nc) as tc, ExitStack() as ctx:
        const_pool = ctx.enter_context(tc.tile_pool(name="const", bufs=1))
        x_pool = ctx.enter_context(tc.tile_pool(name="x", bufs=4 * KD))
        out_pool = ctx.enter_context(tc.tile_pool(name="o", bufs=6))
        psum_pool = ctx.enter_context(tc.tile_pool(name="ps", bufs=6, space="PSUM"))

        const_sb = const_pool.tile([128, CONST_F], mm_dt)
        # bias/ones first (the seed matmul is the first consumer), then wt
        # chunks in parallel (Bacc legalizes multi-wait consumers)
        nc.scalar.dma_start(
            const_sb[0:1, KD * NOUT :], consts[0:1, KD * NOUT :]
        )
        for c in range(KD):
            nc.scalar.dma_start(
                const_sb[:, c * NOUT : (c + 1) * NOUT],
                consts[:, c * NOUT : (c + 1) * NOUT],
            )
        wt_sb = [const_sb[:, c * NOUT : (c + 1) * NOUT] for c in range(KD)]
        bias2_sb = const_sb[0:1, KD * NOUT : KD * NOUT + 2 * NOUT]
        ones_sb = const_sb[0:1, KD * NOUT + 2 * NOUT : KD * NOUT + 2 * NOUT + 128]

        for blk in range(BC // BLK):
            # one tile per D-chunk so each matmul waits on exactly one DMA
            x_sb = []
            for c in range(KD):
                xc = x_pool.tile([128, BLK], mm_dt, tag="x")
                nc.sync.dma_start(xc[:], xT[blk, c])
                x_sb.append(xc)
            for jg in range(BLK // (128 * OJ)):
                # one out tile covers OJ j-subtiles -> one big out-DMA
                ot = out_pool.tile([128, OJ, NOUT], f32, tag="ot")
                for pj in range(OJ // 2):
                    # a pair of j-subtiles shares one full PSUM bank so the
                    # epilogue runs 512-wide (halves per-op access latency)
                    oj0 = pj * 2
                    ps = psum_pool.tile([128, 2, NOUT], f32)
                    # seed both halves with the (doubled) bias row
                    nc.tensor.matmul(
                        ps[:],
                        lhsT=ones_sb,
                        rhs=bias2_sb.rearrange("a (q n) -> a q n", q=2),
                        start=True,
                        stop=False,
                    )
                    for jj in range(2):
                        j = jg * OJ + oj0 + jj
                        for c in range(KD):
                            nc.tensor.matmul(
                                ps[:, jj, :],
                                lhsT=x_sb[c][:, bass.ts(j, 128)],
                                rhs=wt_sb[c],
                                start=False,
                                stop=(jj == 1 and c == KD - 1),
                                skip_group_check=True,
                            )
                    # Scalar LUT set has exp+ln but no softplus/sigmoid:
                    #   off-diag: -softplus(z) = -ln(1 + e^z)
                    #   diag: host negated Wd rows, so psum holds -z and
                    #         sigmoid(z) = 1/(1 + e^-z) = 1/(1 + E_diag)
                    ev = out_pool.tile([128, 2, NOUT], f32, tag="ev")
                    nc.scalar.activation(
                        ev[:], ps[:], mybir.ActivationFunctionType.Exp
                    )
                    nc.scalar.activation(
                        ot[:, oj0 : oj0 + 2, :],
                        ev[:],
                        mybir.ActivationFunctionType.Ln,
                        bias=1.0,
                    )
                    nc.vector.tensor_scalar_mul(
                        ot[:, oj0 : oj0 + 2, :], ot[:, oj0 : oj0 + 2, :], -1.0
                    )
                    dtmp = out_pool.tile([128, 2, K], f32, tag="dtmp")
                    nc.vector.tensor_scalar_add(dtmp[:], ev[:, :, ::17], 1.0)
                    nc.vector.reciprocal(ot[:, oj0 : oj0 + 2, ::17], dtmp[:])
                # dest rows r0+oj*128+p for tile element (p, oj, n)
                r0 = blk * BLK + jg * 128 * OJ
                dst = out[r0 : r0 + 128 * OJ, :].rearrange(
                    "(oj p) n -> p oj n", p=128
                )
                nc.sync.dma_start(dst, ot[:])
    nc.compile()
    return nc


def _get_nc():
    key = MM_DT_NAME
    if key not in _CACHE:
        _CACHE[key] = _build_bass()
    return _CACHE[key]


def _host_prep(x, Wd, bd, Wo, bo):
    np_dt = _np_dt()
    off_i, off_j = np.nonzero(~np.eye(K, dtype=bool))
    w_full = np.empty((NOUT, D), np.float32)
    b_full = np.empty(NOUT, np.float32)
    w_full[off_i * K + off_j] = Wo
    b_full[off_i * K + off_j] = bo
    # diag rows negated: device computes sigmoid(z) as 1/(1 + exp(-z))
    diag_pos = np.arange(K) * (K + 1)
    w_full[diag_pos] = -Wd
    b_full[diag_pos] = -bd
    wt = w_full.T  # (D, 256)
    # const blob layout must match const_sb: [128, CONST_F]
    consts = np.zeros((128, CONST_F), np.float32)
    # wt_sb chunk c at cols [c*256, (c+1)*256): consts[p, c*256+n] = wt[c*128+p, n]
    consts[:, : KD * NOUT] = wt.reshape(KD, 128, NOUT).transpose(1, 0, 2).reshape(
        128, KD * NOUT
    )
    consts[0, KD * NOUT : KD * NOUT + NOUT] = b_full
    consts[0, KD * NOUT + NOUT : KD * NOUT + 2 * NOUT] = b_full
    consts[0, KD * NOUT + 2 * NOUT : KD * NOUT + 2 * NOUT + 128] = 1.0
    consts = np.ascontiguousarray(consts).astype(np_dt)
    nblk = BC // BLK
    in_maps = []
    for c in range(NCORES):
        xs = x[c * BC : (c + 1) * BC]  # (BC, D)
        # -> (nblk, KD, 128, BLK): element (b, kd, p, t) = xs[b*BLK+t, kd*128+p]
        xT = np.ascontiguousarray(
            xs.reshape(nblk, BLK, KD, 128).transpose(0, 2, 3, 1)
        ).astype(np_dt)
        in_maps.append({"xT": xT, "consts": consts})
    return in_maps


def _run(in_maps, **kwargs):
    nc = _get_nc()
    return run_bass_kernel_spmd(nc, in_maps, list(range(NCORES)), **kwargs)


def kernel(x, Wd, bd, Wo, bo, _bench_results=None, **kwargs):
    x = np.asarray(x, np.float32)
    in_maps = _host_prep(
        x,
        np.asarray(Wd, np.float32),
        np.asarray(bd, np.float32),
        np.asarray(Wo, np.float32),
        np.asarray(bo, np.float32),
    )
    res = _run(in_maps, **kwargs)
    if _bench_results is not None:
        _bench_results.append(res)
    outs = [res.results[c]["out"] for c in range(NCORES)]
    return np.concatenate(outs, axis=0).reshape(B, K, K)
